# revision 30
# baseline (speedup 1.0000x reference)
"""Causal self-attention Trainium2 Bass kernel.

Shapes (hardcoded): B=8, T=1024, C=768, NH=12, HS=64.
Sharding: data-parallel over batch — core b computes batch element b.

Per-core dataflow (all matmuls bf16 with fp32 PSUM accumulation):
  - All DRAM operands are pre-tiled on the host so every DMA is 128
    partitions x long contiguous runs; xT and wv are split into per-k
    DMAs so the first projections start as soon as their tiles land.
  - qkT  [2C, T] channel-major  = w_qk.T-tiles (stationary) x xT (moving).
    Pair hp+1's QK projection is emitted between chunk 0 and chunk 1 of
    pair hp. The PSUM->SBUF copies ride the Vector queue, emitted
    between norm stage 1 and stage 2 of chunk 0 so they are not stuck
    behind the full normalize chain.
  - v    token-major [T, C], assembled into v_aug [jt, head, 65] with a
    ones column so the PV matmul also emits softmax row-sums for free
  - S^T  [j, i] blocks per head: lhsT = kT j-tile (K=64), rhs = qT i-cols.
    Causality via block skipping plus an additive -1920 lower-triangular
    constant accumulated into diagonal blocks by a bf16 matmul (id.T @
    mtri); exp(0.125*(S-1920)) underflows to exact 0.
  - exp via ScalarE activation (scale=1/8) PSUM->SBUF into bf16 P^T.
    PV of group g-1 is interleaved between S^T groups.
  - y^T [65, i] = v_aug.T x P^T accumulated over j-tiles in PSUM; row 64
    is the softmax denominator. Norm stage 1 (DVE): yst copy frees the
    PSUM slot, sums-row copy + reciprocal_approx_fast; stage 2: gpsimd
    partition_broadcast + DVE multiply into bf16 yT [C, T]. The last
    chunk skips the yst staging (multiplies straight out of PSUM) to
    shorten the tail dependency chain.
  - out [T, C] = yT-tiles (stationary) x w_proj (moving), DVE copy to
    bf16 [128, it, C] tiles, one DMA per t-tile (host un-tiles and
    casts back to f32). The 16 projection units are interleaved into
    the last pair's chunk-1 emission and the tail.
"""

import numpy as np

import concourse.bass as bass
import concourse.mybir as mybir
import concourse.tile as tile
from concourse import bacc
from concourse.bass_utils import run_bass_kernel_spmd

B, T, C = 8, 1024, 768
NH, HS = 12, 64
NCORES = 8
KT = C // 128            # 6 contraction tiles
NPAIR = NH // 2          # 6 head pairs; head-pair hp covers heads 2hp, 2hp+1
F32 = mybir.dt.float32
BF16 = mybir.dt.bfloat16

_cache = {}


def _build_program(bias_attn: bool, bias_proj: bool):
    nc = bacc.Bacc("TRN2", target_bir_lowering=False, debug=False,
                   num_devices=NCORES)

    # Pre-tiled DRAM layouts (see _prep_inputs): every tensor is
    # [128, ...] with the full free dim contiguous per partition.
    xT = nc.dram_tensor("xT", [128, KT * T], BF16, kind="ExternalInput")
    wqk = nc.dram_tensor("wqk", [128, NPAIR * KT * 2 * 128], BF16,
                         kind="ExternalInput")
    wv = nc.dram_tensor("wv", [128, KT * C], BF16, kind="ExternalInput")
    wp = nc.dram_tensor("wp", [128, KT * C], BF16, kind="ExternalInput")
    if bias_attn:
        bqk_d = nc.dram_tensor("bqk", [2 * C], F32, kind="ExternalInput")
        bv_d = nc.dram_tensor("bv", [C], F32, kind="ExternalInput")
    if bias_proj:
        bp_d = nc.dram_tensor("bp", [C], F32, kind="ExternalInput")
    out = nc.dram_tensor("out", [128, 8 * C], BF16, kind="ExternalOutput")
    out_r = out.ap().rearrange("p (i m) -> p i m", m=C)

    # Additive causal mask for diagonal S^T blocks, applied as a bf16
    # accumulation matmul (identity.T @ mtri adds mtri to the PSUM tile):
    # -1920 pre-scale = -240 post-scale -> exp underflows to exactly 0.
    import ml_dtypes
    mtri_np = (np.tril(np.ones((128, 128)), k=-1) * -1920.0).astype(
        ml_dtypes.bfloat16)
    # two copies side by side: one 256-col matmul masks both diagonal
    # blocks of an S^T group (out AP strides across the two u-strips)
    mtri2_d = nc.inline_tensor(
        np.concatenate([mtri_np, mtri_np], axis=1), "mtri2")
    id_np = np.eye(128, dtype=ml_dtypes.bfloat16)
    id_d = nc.inline_tensor(id_np, "id128")

    xT_r = xT.ap().rearrange("p (k t) -> p k t", t=T)
    wqk_r = wqk.ap().rearrange("p (hp k u m) -> p hp k u m", k=KT, u=2, m=128)
    wv_r = wv.ap().rearrange("p (k m) -> p k m", m=C)
    wp_r = wp.ap().rearrange("p (k m) -> p k m", m=C)

    with tile.TileContext(nc) as tc:
        with (
            tc.tile_pool(name="xpool", bufs=1) as xpool,
            tc.tile_pool(name="cpool", bufs=1) as cpool,
            tc.tile_pool(name="wvpool", bufs=1) as wvpool,
            tc.tile_pool(name="vpool", bufs=1) as vpool,
            tc.tile_pool(name="wqkpool", bufs=3) as wqkpool,
            tc.tile_pool(name="qkpool", bufs=3) as qkpool,
            tc.tile_pool(name="ptpool", bufs=6) as ptpool,
            tc.tile_pool(name="ytpool", bufs=1) as ytpool,
            tc.tile_pool(name="wppool", bufs=1) as wppool,
            tc.tile_pool(name="opool", bufs=3) as opool,
            tc.tile_pool(name="smpool", bufs=6) as smpool,
            tc.tile_pool(name="psA", bufs=3, space="PSUM") as psA,
            tc.tile_pool(name="psB", bufs=2, space="PSUM") as psB,
        ):
            # ---- tiny constants first, then first-pair weights, then
            # x: the first QK matmul can start as soon as wt_0 + xT
            # k-tile 0 land ----
            mtri2_s = cpool.tile([128, 256], BF16, tag="mtri2")
            nc.sync.dma_start(mtri2_s[:], mtri2_d.ap())
            id_s = cpool.tile([128, 128], BF16, tag="id128")
            nc.sync.dma_start(id_s[:], id_d.ap())

            def emit_qkproj_dma(hp):
                wt = wqkpool.tile([128, KT, 2, 128], BF16, tag="wqk",
                                  name=f"wt_{hp}")
                nc.sync.dma_start(wt[:], wqk_r[:, hp])
                return wt

            wt_next = emit_qkproj_dma(0)

            xT_s = xpool.tile([128, KT, T], BF16, tag="xT")
            for k in range(KT):
                for h in range(2):
                    nc.sync.dma_start(xT_s[:, k, h * 512:(h + 1) * 512],
                                      xT_r[:, k, h * 512:(h + 1) * 512])
            # Chain wv behind xT (dummy copy creates the dependency) so
            # its transfer does not steal HBM bandwidth from the
            # critical-path xT tiles; likewise wpt behind wv.
            wv_s = wvpool.tile([128, KT, C], BF16, tag="wv")
            nc.vector.tensor_copy(wv_s[0:1, KT - 1, C - 1:C],
                                  xT_s[0:1, KT - 1, T - 1:T])
            for k in range(KT):
                nc.sync.dma_start(wv_s[:, k, :], wv_r[:, k, :])

            # ---- bias staging ----
            if bias_attn:
                bqk_s = cpool.tile([128, 12], F32, tag="bqk")
                nc.sync.dma_start(bqk_s[:], bqk_d.ap().rearrange("(m p) -> p m", p=128))
                bv_row = cpool.tile([1, C], F32, tag="bvrow")
                nc.sync.dma_start(bv_row[:], bv_d.ap().rearrange("c -> 1 c"))
                bv_bc = cpool.tile([128, C], F32, tag="bvbc")
                nc.gpsimd.partition_broadcast(bv_bc[:], bv_row[:])
            if bias_proj:
                bp_row = cpool.tile([1, C], F32, tag="bprow")
                nc.sync.dma_start(bp_row[:], bp_d.ap().rearrange("c -> 1 c"))
                bp_bc = cpool.tile([128, C], F32, tag="bpbc")
                nc.gpsimd.partition_broadcast(bp_bc[:], bp_row[:])



            def emit_qkproj(hp, wt):
                qk_t = qkpool.tile([128, 2, T], BF16, tag="qk",
                                   name=f"qk_{hp}")
                for part in range(2):  # 0 = q m-tile hp, 1 = k m-tile hp
                    ps = psA.tile([128, 1024], F32, tag="big",
                                  name=f"qkps_{hp}_{part}")
                    # Per-half PSUM->SBUF copies on DVE directly behind
                    # each half's matmuls: by the time the in-order DVE
                    # queue reaches a copy its dep is satisfied, so no
                    # wait-queue parking and the PSUM slot frees ~one
                    # half earlier (the next chunk's S^T reuses it).
                    for nch in range(2):
                        sl = slice(nch * 512, (nch + 1) * 512)
                        for k in range(KT):
                            nc.tensor.matmul(
                                ps[:, sl],
                                wt[:, k, part, :],
                                xT_s[:, k, sl],
                                start=(k == 0), stop=(k == KT - 1),
                            )
                        if bias_attn:
                            nc.vector.tensor_add(
                                qk_t[:, part, sl], ps[:, sl],
                                bqk_s[:, part * 6 + hp:part * 6 + hp + 1])
                        else:
                            nc.vector.tensor_copy(qk_t[:, part, sl],
                                                  ps[:, sl])
                return qk_t

            qk_next = emit_qkproj(0, wt_next)

            # ---- V: token-major, assembled as v_aug[jt, head, 128]
            # with cols 64-127 all-ones: the PV matmul then emits 64
            # identical softmax-denominator rows into PSUM partitions
            # 64-127 for free (matmul time is independent of output
            # partition count), so the normalize chain runs 64-wide
            # with no partition_broadcast.  ----
            v_aug = vpool.tile([128, 8, NH, 128], BF16, tag="vaug")
            nc.vector.memset(v_aug[:, :, :, HS:128], 1.0)
            for jt in range(8):
                ps = psA.tile([128, 1024], F32, tag="big")
                for off, w in ((0, 512), (512, 256)):
                    for k in range(KT):
                        nc.tensor.matmul(
                            ps[:, off:off + w],
                            xT_s[:, k, jt * 128:(jt + 1) * 128],
                            wv_s[:, k, off:off + w],
                            start=(k == 0), stop=(k == KT - 1),
                        )
                dst = v_aug[:, jt, :, 0:HS]
                src = ps[:, 0:C].rearrange("p (h d) -> p h d", d=HS)
                if bias_attn:
                    nc.vector.tensor_add(
                        dst, src, bv_bc[:].rearrange("p (h d) -> p h d", d=HS))
                else:
                    nc.vector.tensor_copy(dst, src)

            # ---- yT accumulator (written during attention) ----
            yT_s = ytpool.tile([128, KT, T], BF16, tag="yT")

            # prefetch projection weights (consumed only at the tail);
            # chained behind wv to keep early HBM bandwidth for x
            wpt = wppool.tile([128, KT, C], BF16, tag="wp")
            nc.vector.tensor_copy(wpt[0:1, KT - 1, C - 1:C],
                                  wv_s[0:1, KT - 1, C - 1:C])
            nc.sync.dma_start(wpt[:], wp_r)

            # ---- output projection, one (t-tile, off-chunk) unit at a
            # time so units can be interleaved into the last pair ----
            ots = {}

            def emit_outproj_unit(it, off, w, on_scalar=False):
                ps = psA.tile([128, 1024], F32, tag="big",
                              name=f"ops_{it}_{off}")
                for k in range(KT):
                    nc.tensor.matmul(
                        ps[:, 0:w],
                        yT_s[:, k, it * 128:(it + 1) * 128],
                        wpt[:, k, off:off + w],
                        start=(k == 0), stop=(k == KT - 1),
                    )
                if it not in ots:
                    ots[it] = opool.tile([128, C], BF16, tag="ot",
                                         name=f"ot_{it}")
                ot = ots[it]
                if bias_proj:
                    nc.vector.tensor_add(ot[:, off:off + w], ps[:, 0:w],
                                         bp_bc[:, off:off + w])
                elif on_scalar:
                    # tail units: ACT is idle after the last exp
                    nc.scalar.copy(ot[:, off:off + w], ps[:, 0:w])
                else:
                    nc.vector.tensor_copy(ot[:, off:off + w], ps[:, 0:w])
                if off + w == C:
                    nc.sync.dma_start(out_r[:, it, :], ot[:])

            # ---- attention core: S^T, exp, PV for one (pair, chunk) ----
            def emit_attn_core(hp, qk_t, c, filler=None, pre_g=None):
                njt = 4 * (c + 1)
                pts = [[ptpool.tile([128, 4, 512], BF16, tag="pt",
                                    name=f"pt_{hp}_{hl}_{c}_{i}")
                        for i in range(njt // 4)] for hl in range(2)]

                def pt_ap(hl, jt):
                    return pts[hl][jt // 4][:, jt % 4, :]

                y_pss = [psB.tile([128, 512], F32, tag="y",
                                  name=f"yps_{hp}_{hl}_{c}")
                         for hl in range(2)]

                def emit_pv(g):
                    # PV for the two j-tiles of group g (both heads)
                    for u in range(2):
                        jt = 2 * g + u
                        lo = max(0, (jt - 4 * c) * 128)
                        for hl in range(2):
                            nc.tensor.matmul(
                                y_pss[hl][:, lo:512],
                                v_aug[:, jt, 2 * hp + hl, :],
                                pt_ap(hl, jt)[:, lo:512],
                                start=(jt == 0),
                                stop=(jt == njt - 1),
                                skip_group_check=(jt > 0),
                            )

                # Software-pipelined emission: PV of group g-2 is
                # interleaved between S^T groups so the in-order PE
                # queue always has work that does not wait on the
                # ScalarE exp (which drains the S^T PSUM slots) and the
                # first PV lands after the previous chunk's norm frees
                # the y PSUM slot.
                LAG = 2
                for g in range(njt // 2):
                    if pre_g is not None and g in pre_g:
                        pre_g[g]()
                    sts = [psA.tile([128, 1024], F32, tag="big",
                                    name=f"st_{hp}_{hl}_{c}_{g}")
                           for hl in range(2)]
                    diag_g = 2 * g >= 4 * c
                    for u in range(2):
                        jt = 2 * g + u
                        lo = max(0, (jt - 4 * c) * 128)
                        for hl in range(2):
                            base = 64 * hl
                            nc.tensor.matmul(
                                sts[hl][:, u * 512 + lo:(u + 1) * 512],
                                qk_t[base:base + 64, 1,
                                     jt * 128:(jt + 1) * 128],
                                qk_t[base:base + 64, 0,
                                     c * 512 + lo:(c + 1) * 512],
                                start=True, stop=not diag_g,
                            )
                    if diag_g:
                        # accumulate -1920 on the j>i triangles of both
                        # diagonal blocks with one 256-col matmul: the
                        # u0 block sits at sub-block b0, the u1 block 5
                        # sub-blocks later (512/128 + 1)
                        b0 = 2 * g - 4 * c
                        for hl in range(2):
                            dst = sts[hl][:].rearrange(
                                "p (x n) -> p x n", n=128)[:, b0:b0 + 6:5, :]
                            nc.tensor.matmul(
                                dst, id_s[:],
                                mtri2_s[:].rearrange("p (u n) -> p u n",
                                                     n=128),
                                start=False, stop=True,
                                skip_group_check=True,
                            )
                    # exp only the live columns [lo(2g):512] of both
                    # strips — columns left of the first j-tile's lo
                    # are never read by PV
                    logp = max(0, (2 * g - 4 * c)) * 128
                    for hl in range(2):
                        nc.scalar.activation(
                            pts[hl][g // 2][:, (g % 2) * 2:(g % 2) * 2 + 2,
                                            logp:512],
                            sts[hl][:].rearrange("p (a n) -> p a n",
                                                 n=512)[:, :, logp:512],
                            mybir.ActivationFunctionType.Exp,
                            scale=0.125,
                        )
                    if g >= LAG:
                        emit_pv(g - LAG)
                    if filler is not None:
                        filler()
                for gg in range(max(0, njt // 2 - LAG), njt // 2):
                    emit_pv(gg)
                return y_pss

            # ---- normalize: yT[h rows, c cols] = y / sums ----
            # Stage 1 (DVE): yst copy frees the y PSUM slot; sums-row
            # copy from PSUM (shifts partition 64 -> 0; the SBUF->SBUF
            # shifted copy reads the wrong lane) and reciprocal in
            # place (custom-DVE needs base partition 0).
            # normalize: yT = y / sums, all on DVE, 64 lanes wide (the
            # PV matmul put 64 copies of the sums in PSUM rows 64-127).
            def emit_norm(hp, c, y_pss, stage_y=True):
                ysts, srows = [], []
                for hl in range(2):
                    if stage_y:
                        yst = smpool.tile([64, 512], F32, tag="yst",
                                          name=f"yst_{hp}_{hl}_{c}")
                        nc.vector.tensor_copy(yst[:], y_pss[hl][0:HS, :])
                    else:
                        yst = None
                    ysts.append(yst)
                for hl in range(2):
                    srow = smpool.tile([64, 512], F32, tag="srow",
                                       name=f"srow_{hp}_{hl}_{c}")
                    nc.vector.tensor_copy(srow[:], y_pss[hl][HS:128, :])
                    nc.vector.reciprocal_approx_fast(srow[:], srow[:])
                    srows.append(srow)
                for hl in range(2):
                    src = (ysts[hl][:] if ysts[hl] is not None
                           else y_pss[hl][0:HS, :])
                    nc.vector.tensor_mul(
                        yT_s[64 * hl:64 * hl + 64, hp,
                             c * 512:(c + 1) * 512],
                        src, srows[hl][:])

            # Output-projection units for t-tiles 0-3 only need chunk-0
            # yT columns (all pairs), so they can slot into the last
            # pair's chunk-1 emission; t-tiles 4-7 run at the tail.
            op_units = [(it, off, w) for it in range(8)
                        for off, w in ((0, 512), (512, 256))]
            fill_iter = iter(op_units[:8])

            def filler():
                u = next(fill_iter, None)
                if u is not None:
                    emit_outproj_unit(*u)

            for hp in range(NPAIR):
                qk_t = qk_next
                last = hp == NPAIR - 1
                if not last:
                    wt_next = emit_qkproj_dma(hp + 1)
                y0 = emit_attn_core(hp, qk_t, 0)
                if not last:
                    qk_next = emit_qkproj(hp + 1, wt_next)

                    # norm of chunk 0 is emitted inside chunk 1 (g=1):
                    # if it were emitted first, its DVE ops would jump
                    # ahead of the parked qk copies in the DVE wait
                    # queue and delay the PSUM recycle.
                    def norm0(hp=hp, y0=y0):
                        emit_norm(hp, 0, y0)

                    y1 = emit_attn_core(hp, qk_t, 1, pre_g={1: norm0})
                else:
                    emit_norm(hp, 0, y0)
                    y1 = emit_attn_core(hp, qk_t, 1, filler=filler)
                emit_norm(hp, 1, y1, stage_y=not last)

            for u in fill_iter:
                emit_outproj_unit(*u, on_scalar=True)
            for u in op_units[8:]:
                emit_outproj_unit(*u, on_scalar=True)

    nc.compile()
    return nc


def _get_program(bias_attn, bias_proj):
    key = (bias_attn, bias_proj)
    if key not in _cache:
        _cache[key] = _build_program(bias_attn, bias_proj)
    return _cache[key]


def _prep_inputs(x, w_attn, b_attn, w_proj, b_proj):
    x = np.asarray(x, dtype=np.float32)
    w_attn = np.asarray(w_attn, dtype=np.float32)
    b_attn = np.asarray(b_attn, dtype=np.float32)
    w_proj = np.asarray(w_proj, dtype=np.float32)
    b_proj = np.asarray(b_proj, dtype=np.float32)
    bias_attn = bool(np.any(b_attn))
    bias_proj = bool(np.any(b_proj))
    import ml_dtypes
    bf = ml_dtypes.bfloat16

    # Pre-tile to [128, free-contiguous] layouts (see _build_program).
    # wqk[p, hp, k, u, m] = w_attn[k*128+p, u*C + hp*128 + m]
    wqk = np.ascontiguousarray(
        w_attn[:, :2 * C].reshape(KT, 128, 2, NPAIR, 128)
        .transpose(1, 3, 0, 2, 4).reshape(128, -1)).astype(bf)
    # wv[p, k, m] = w_attn[k*128+p, 2C + m]
    wv = np.ascontiguousarray(
        w_attn[:, 2 * C:].reshape(KT, 128, C)
        .transpose(1, 0, 2).reshape(128, -1)).astype(bf)
    # wp[p, k, m] = w_proj[k*128+p, m]
    wpb = np.ascontiguousarray(
        w_proj.reshape(KT, 128, C).transpose(1, 0, 2).reshape(128, -1)
    ).astype(bf)
    in_maps = []
    for b in range(NCORES):
        # xT[p, k, t] = x[b, t, k*128+p]
        xt = np.ascontiguousarray(
            x[b].T.reshape(KT, 128, T).transpose(1, 0, 2).reshape(128, -1)
        ).astype(bf)
        m = {
            "xT": xt,
            "wqk": wqk,
            "wv": wv,
            "wp": wpb,
        }
        if bias_attn:
            m["bqk"] = np.ascontiguousarray(b_attn[:2 * C])
            m["bv"] = np.ascontiguousarray(b_attn[2 * C:])
        if bias_proj:
            m["bp"] = b_proj
        in_maps.append(m)
    return in_maps, bias_attn, bias_proj


def run(x, w_attn, b_attn, w_proj, b_proj, trace=False, tmpdir=None):
    in_maps, bias_attn, bias_proj = _prep_inputs(
        x, w_attn, b_attn, w_proj, b_proj)
    nc = _get_program(bias_attn, bias_proj)
    res = run_bass_kernel_spmd(nc, in_maps, list(range(NCORES)),
                               trace=trace, tmpdir=tmpdir)
    # out[p, it, m] -> [T, C]
    out = np.stack(
        [np.ascontiguousarray(
            res.results[i]["out"].reshape(128, 8, C).transpose(1, 0, 2)
            .reshape(T, C)) for i in range(NCORES)], axis=0)
    return out.astype(np.float32), res


def kernel(x, w_attn, b_attn, w_proj, b_proj):
    out, _ = run(x, w_attn, b_attn, w_proj, b_proj)
    return out


# revision 31
# speedup vs baseline: 1.0070x; 1.0070x over previous
"""Causal self-attention Trainium2 Bass kernel.

Shapes (hardcoded): B=8, T=1024, C=768, NH=12, HS=64.
Sharding: data-parallel over batch — core b computes batch element b.

Per-core dataflow (all matmuls bf16 with fp32 PSUM accumulation):
  - All DRAM operands are pre-tiled on the host so every DMA is 128
    partitions x long contiguous runs; xT and wv are split into per-k
    DMAs so the first projections start as soon as their tiles land.
  - qkT  [2C, T] channel-major  = w_qk.T-tiles (stationary) x xT (moving).
    Pair hp+1's QK projection is emitted between chunk 0 and chunk 1 of
    pair hp. The PSUM->SBUF copies ride the Vector queue, emitted
    between norm stage 1 and stage 2 of chunk 0 so they are not stuck
    behind the full normalize chain.
  - v    token-major [T, C], assembled into v_aug [jt, head, 65] with a
    ones column so the PV matmul also emits softmax row-sums for free
  - S^T  [j, i] blocks per head: lhsT = kT j-tile (K=64), rhs = qT i-cols.
    Causality via block skipping plus an additive -1920 lower-triangular
    constant accumulated into diagonal blocks by a bf16 matmul (id.T @
    mtri); exp(0.125*(S-1920)) underflows to exact 0.
  - exp via ScalarE activation (scale=1/8) PSUM->SBUF into bf16 P^T.
    PV of group g-1 is interleaved between S^T groups.
  - y^T [65, i] = v_aug.T x P^T accumulated over j-tiles in PSUM; row 64
    is the softmax denominator. Norm stage 1 (DVE): yst copy frees the
    PSUM slot, sums-row copy + reciprocal_approx_fast; stage 2: gpsimd
    partition_broadcast + DVE multiply into bf16 yT [C, T]. The last
    chunk skips the yst staging (multiplies straight out of PSUM) to
    shorten the tail dependency chain.
  - out [T, C] = yT-tiles (stationary) x w_proj (moving), DVE copy to
    bf16 [128, it, C] tiles, one DMA per t-tile (host un-tiles and
    casts back to f32). The 16 projection units are interleaved into
    the last pair's chunk-1 emission and the tail.
"""

import numpy as np

import concourse.bass as bass
import concourse.mybir as mybir
import concourse.tile as tile
from concourse import bacc
from concourse.bass_utils import run_bass_kernel_spmd

B, T, C = 8, 1024, 768
NH, HS = 12, 64
NCORES = 8
KT = C // 128            # 6 contraction tiles
NPAIR = NH // 2          # 6 head pairs; head-pair hp covers heads 2hp, 2hp+1
F32 = mybir.dt.float32
BF16 = mybir.dt.bfloat16

_cache = {}


def _build_program(bias_attn: bool, bias_proj: bool):
    nc = bacc.Bacc("TRN2", target_bir_lowering=False, debug=False,
                   num_devices=NCORES)

    # Pre-tiled DRAM layouts (see _prep_inputs): every tensor is
    # [128, ...] with the full free dim contiguous per partition.
    xT = nc.dram_tensor("xT", [128, KT * T], BF16, kind="ExternalInput")
    wqk = nc.dram_tensor("wqk", [128, NPAIR * KT * 2 * 128], BF16,
                         kind="ExternalInput")
    wv = nc.dram_tensor("wv", [128, KT * C], BF16, kind="ExternalInput")
    wp = nc.dram_tensor("wp", [128, KT * C], BF16, kind="ExternalInput")
    if bias_attn:
        bqk_d = nc.dram_tensor("bqk", [2 * C], F32, kind="ExternalInput")
        bv_d = nc.dram_tensor("bv", [C], F32, kind="ExternalInput")
    if bias_proj:
        bp_d = nc.dram_tensor("bp", [C], F32, kind="ExternalInput")
    out = nc.dram_tensor("out", [128, 8 * C], BF16, kind="ExternalOutput")
    out_r = out.ap().rearrange("p (i m) -> p i m", m=C)

    # Additive causal mask for diagonal S^T blocks, applied as a bf16
    # accumulation matmul (identity.T @ mtri adds mtri to the PSUM tile):
    # -1920 pre-scale = -240 post-scale -> exp underflows to exactly 0.
    import ml_dtypes
    mtri_np = (np.tril(np.ones((128, 128)), k=-1) * -1920.0).astype(
        ml_dtypes.bfloat16)
    # two copies side by side: one 256-col matmul masks both diagonal
    # blocks of an S^T group (out AP strides across the two u-strips)
    mtri2_d = nc.inline_tensor(
        np.concatenate([mtri_np, mtri_np], axis=1), "mtri2")
    id_np = np.eye(128, dtype=ml_dtypes.bfloat16)
    id_d = nc.inline_tensor(id_np, "id128")

    xT_r = xT.ap().rearrange("p (k t) -> p k t", t=T)
    wqk_r = wqk.ap().rearrange("p (hp k u m) -> p hp k u m", k=KT, u=2, m=128)
    wv_r = wv.ap().rearrange("p (k m) -> p k m", m=C)
    wp_r = wp.ap().rearrange("p (k m) -> p k m", m=C)

    with tile.TileContext(nc) as tc:
        with (
            tc.tile_pool(name="xpool", bufs=1) as xpool,
            tc.tile_pool(name="cpool", bufs=1) as cpool,
            tc.tile_pool(name="wvpool", bufs=1) as wvpool,
            tc.tile_pool(name="vpool", bufs=1) as vpool,
            tc.tile_pool(name="wqkpool", bufs=3) as wqkpool,
            tc.tile_pool(name="qkpool", bufs=3) as qkpool,
            tc.tile_pool(name="ptpool", bufs=6) as ptpool,
            tc.tile_pool(name="ytpool", bufs=1) as ytpool,
            tc.tile_pool(name="wppool", bufs=1) as wppool,
            tc.tile_pool(name="opool", bufs=3) as opool,
            tc.tile_pool(name="smpool", bufs=6) as smpool,
            tc.tile_pool(name="psA", bufs=3, space="PSUM") as psA,
            tc.tile_pool(name="psB", bufs=2, space="PSUM") as psB,
        ):
            # ---- tiny constants first, then first-pair weights, then
            # x: the first QK matmul can start as soon as wt_0 + xT
            # k-tile 0 land ----
            mtri2_s = cpool.tile([128, 256], BF16, tag="mtri2")
            nc.sync.dma_start(mtri2_s[:], mtri2_d.ap())
            id_s = cpool.tile([128, 128], BF16, tag="id128")
            nc.sync.dma_start(id_s[:], id_d.ap())

            def emit_qkproj_dma(hp):
                wt = wqkpool.tile([128, KT, 2, 128], BF16, tag="wqk",
                                  name=f"wt_{hp}")
                nc.sync.dma_start(wt[:], wqk_r[:, hp])
                return wt

            wt_next = emit_qkproj_dma(0)

            xT_s = xpool.tile([128, KT, T], BF16, tag="xT")
            for k in range(KT):
                for h in range(2):
                    nc.sync.dma_start(xT_s[:, k, h * 512:(h + 1) * 512],
                                      xT_r[:, k, h * 512:(h + 1) * 512])
            # Chain wv behind xT (dummy copy creates the dependency) so
            # its transfer does not steal HBM bandwidth from the
            # critical-path xT tiles; likewise wpt behind wv.
            wv_s = wvpool.tile([128, KT, C], BF16, tag="wv")
            nc.vector.tensor_copy(wv_s[0:1, KT - 1, C - 1:C],
                                  xT_s[0:1, KT - 1, T - 1:T])
            for k in range(KT):
                nc.sync.dma_start(wv_s[:, k, :], wv_r[:, k, :])

            # ---- bias staging ----
            if bias_attn:
                bqk_s = cpool.tile([128, 12], F32, tag="bqk")
                nc.sync.dma_start(bqk_s[:], bqk_d.ap().rearrange("(m p) -> p m", p=128))
                bv_row = cpool.tile([1, C], F32, tag="bvrow")
                nc.sync.dma_start(bv_row[:], bv_d.ap().rearrange("c -> 1 c"))
                bv_bc = cpool.tile([128, C], F32, tag="bvbc")
                nc.gpsimd.partition_broadcast(bv_bc[:], bv_row[:])
            if bias_proj:
                bp_row = cpool.tile([1, C], F32, tag="bprow")
                nc.sync.dma_start(bp_row[:], bp_d.ap().rearrange("c -> 1 c"))
                bp_bc = cpool.tile([128, C], F32, tag="bpbc")
                nc.gpsimd.partition_broadcast(bp_bc[:], bp_row[:])



            def emit_qkproj(hp, wt):
                qk_t = qkpool.tile([128, 2, T], BF16, tag="qk",
                                   name=f"qk_{hp}")
                for part in range(2):  # 0 = q m-tile hp, 1 = k m-tile hp
                    ps = psA.tile([128, 1024], F32, tag="big",
                                  name=f"qkps_{hp}_{part}")
                    # Per-half PSUM->SBUF copies on DVE directly behind
                    # each half's matmuls: by the time the in-order DVE
                    # queue reaches a copy its dep is satisfied, so no
                    # wait-queue parking and the PSUM slot frees ~one
                    # half earlier (the next chunk's S^T reuses it).
                    for nch in range(2):
                        sl = slice(nch * 512, (nch + 1) * 512)
                        for k in range(KT):
                            nc.tensor.matmul(
                                ps[:, sl],
                                wt[:, k, part, :],
                                xT_s[:, k, sl],
                                start=(k == 0), stop=(k == KT - 1),
                            )
                        if bias_attn:
                            nc.vector.tensor_add(
                                qk_t[:, part, sl], ps[:, sl],
                                bqk_s[:, part * 6 + hp:part * 6 + hp + 1])
                        else:
                            nc.vector.tensor_copy(qk_t[:, part, sl],
                                                  ps[:, sl])
                return qk_t

            qk_next = emit_qkproj(0, wt_next)

            # ---- V: token-major, assembled as v_aug[jt, head, 128]
            # with cols 64-127 all-ones: the PV matmul then emits 64
            # identical softmax-denominator rows into PSUM partitions
            # 64-127 for free (matmul time is independent of output
            # partition count), so the normalize chain runs 64-wide
            # with no partition_broadcast.  ----
            v_aug = vpool.tile([128, 8, NH, 128], BF16, tag="vaug")
            nc.vector.memset(v_aug[:, :, :, HS:128], 1.0)
            for jt in range(8):
                ps = psA.tile([128, 1024], F32, tag="big")
                for off, w in ((0, 512), (512, 256)):
                    for k in range(KT):
                        nc.tensor.matmul(
                            ps[:, off:off + w],
                            xT_s[:, k, jt * 128:(jt + 1) * 128],
                            wv_s[:, k, off:off + w],
                            start=(k == 0), stop=(k == KT - 1),
                        )
                dst = v_aug[:, jt, :, 0:HS]
                src = ps[:, 0:C].rearrange("p (h d) -> p h d", d=HS)
                if bias_attn:
                    nc.vector.tensor_add(
                        dst, src, bv_bc[:].rearrange("p (h d) -> p h d", d=HS))
                else:
                    nc.vector.tensor_copy(dst, src)

            # ---- yT accumulator (written during attention) ----
            yT_s = ytpool.tile([128, KT, T], BF16, tag="yT")

            # prefetch projection weights (consumed only at the tail);
            # chained behind wv to keep early HBM bandwidth for x
            wpt = wppool.tile([128, KT, C], BF16, tag="wp")
            nc.vector.tensor_copy(wpt[0:1, KT - 1, C - 1:C],
                                  wv_s[0:1, KT - 1, C - 1:C])
            nc.sync.dma_start(wpt[:], wp_r)

            # ---- output projection, one (t-tile, off-chunk) unit at a
            # time so units can be interleaved into the last pair ----
            ots = {}

            def emit_outproj_unit(it, off, w, on_scalar=False):
                ps = psA.tile([128, 1024], F32, tag="big",
                              name=f"ops_{it}_{off}")
                for k in range(KT):
                    nc.tensor.matmul(
                        ps[:, 0:w],
                        yT_s[:, k, it * 128:(it + 1) * 128],
                        wpt[:, k, off:off + w],
                        start=(k == 0), stop=(k == KT - 1),
                    )
                if it not in ots:
                    ots[it] = opool.tile([128, C], BF16, tag="ot",
                                         name=f"ot_{it}")
                ot = ots[it]
                if bias_proj:
                    nc.vector.tensor_add(ot[:, off:off + w], ps[:, 0:w],
                                         bp_bc[:, off:off + w])
                elif on_scalar:
                    # tail units: ACT is idle after the last exp
                    nc.scalar.copy(ot[:, off:off + w], ps[:, 0:w])
                else:
                    nc.vector.tensor_copy(ot[:, off:off + w], ps[:, 0:w])
                if off + w == C:
                    nc.sync.dma_start(out_r[:, it, :], ot[:])

            # ---- attention core: S^T, exp, PV for one (pair, chunk) ----
            def emit_attn_core(hp, qk_t, c, filler=None, pre_g=None):
                njt = 4 * (c + 1)
                pts = [[ptpool.tile([128, 4, 512], BF16, tag="pt",
                                    name=f"pt_{hp}_{hl}_{c}_{i}")
                        for i in range(njt // 4)] for hl in range(2)]

                def pt_ap(hl, jt):
                    return pts[hl][jt // 4][:, jt % 4, :]

                y_pss = [psB.tile([128, 512], F32, tag="y",
                                  name=f"yps_{hp}_{hl}_{c}")
                         for hl in range(2)]

                def emit_pv(g):
                    # PV for the two j-tiles of group g (both heads)
                    for u in range(2):
                        jt = 2 * g + u
                        lo = max(0, (jt - 4 * c) * 128)
                        for hl in range(2):
                            nc.tensor.matmul(
                                y_pss[hl][:, lo:512],
                                v_aug[:, jt, 2 * hp + hl, :],
                                pt_ap(hl, jt)[:, lo:512],
                                start=(jt == 0),
                                stop=(jt == njt - 1),
                                skip_group_check=(jt > 0),
                            )

                # Software-pipelined emission: PV of group g-2 is
                # interleaved between S^T groups so the in-order PE
                # queue always has work that does not wait on the
                # ScalarE exp (which drains the S^T PSUM slots) and the
                # first PV lands after the previous chunk's norm frees
                # the y PSUM slot.
                LAG = 2
                for g in range(njt // 2):
                    if pre_g is not None and g in pre_g:
                        pre_g[g]()
                    sts = [psA.tile([128, 1024], F32, tag="big",
                                    name=f"st_{hp}_{hl}_{c}_{g}")
                           for hl in range(2)]
                    diag_g = 2 * g >= 4 * c
                    logp = max(0, (2 * g - 4 * c)) * 128
                    # heads sequential (not interleaved): head hl's exp
                    # is emitted before head hl+1's S^T so it starts ~1us
                    # earlier and its PSUM slot recycles sooner
                    for hl in range(2):
                        base = 64 * hl
                        for u in range(2):
                            jt = 2 * g + u
                            lo = max(0, (jt - 4 * c) * 128)
                            nc.tensor.matmul(
                                sts[hl][:, u * 512 + lo:(u + 1) * 512],
                                qk_t[base:base + 64, 1,
                                     jt * 128:(jt + 1) * 128],
                                qk_t[base:base + 64, 0,
                                     c * 512 + lo:(c + 1) * 512],
                                start=True, stop=not diag_g,
                            )
                        if diag_g:
                            # accumulate -1920 on the j>i triangles of
                            # both diagonal blocks with one 256-col
                            # matmul: the u0 block sits at sub-block b0,
                            # the u1 block 5 sub-blocks later
                            b0 = 2 * g - 4 * c
                            dst = sts[hl][:].rearrange(
                                "p (x n) -> p x n", n=128)[:, b0:b0 + 6:5, :]
                            nc.tensor.matmul(
                                dst, id_s[:],
                                mtri2_s[:].rearrange("p (u n) -> p u n",
                                                     n=128),
                                start=False, stop=True,
                                skip_group_check=True,
                            )
                        # exp only the live columns [lo(2g):512] of both
                        # strips — columns left of the first j-tile's lo
                        # are never read by PV
                        nc.scalar.activation(
                            pts[hl][g // 2][:, (g % 2) * 2:(g % 2) * 2 + 2,
                                            logp:512],
                            sts[hl][:].rearrange("p (a n) -> p a n",
                                                 n=512)[:, :, logp:512],
                            mybir.ActivationFunctionType.Exp,
                            scale=0.125,
                        )
                    if g >= LAG:
                        emit_pv(g - LAG)
                    if filler is not None:
                        filler()
                for gg in range(max(0, njt // 2 - LAG), njt // 2):
                    emit_pv(gg)
                return y_pss

            # ---- normalize: yT[h rows, c cols] = y / sums ----
            # Stage 1 (DVE): yst copy frees the y PSUM slot; sums-row
            # copy from PSUM (shifts partition 64 -> 0; the SBUF->SBUF
            # shifted copy reads the wrong lane) and reciprocal in
            # place (custom-DVE needs base partition 0).
            # normalize: yT = y / sums, all on DVE, 64 lanes wide (the
            # PV matmul put 64 copies of the sums in PSUM rows 64-127).
            def emit_norm(hp, c, y_pss, stage_y=True):
                ysts, srows = [], []
                for hl in range(2):
                    if stage_y:
                        yst = smpool.tile([64, 512], F32, tag="yst",
                                          name=f"yst_{hp}_{hl}_{c}")
                        nc.vector.tensor_copy(yst[:], y_pss[hl][0:HS, :])
                    else:
                        yst = None
                    ysts.append(yst)
                for hl in range(2):
                    srow = smpool.tile([64, 512], F32, tag="srow",
                                       name=f"srow_{hp}_{hl}_{c}")
                    nc.vector.tensor_copy(srow[:], y_pss[hl][HS:128, :])
                    nc.vector.reciprocal_approx_fast(srow[:], srow[:])
                    srows.append(srow)
                for hl in range(2):
                    src = (ysts[hl][:] if ysts[hl] is not None
                           else y_pss[hl][0:HS, :])
                    nc.vector.tensor_mul(
                        yT_s[64 * hl:64 * hl + 64, hp,
                             c * 512:(c + 1) * 512],
                        src, srows[hl][:])

            # Output-projection units for t-tiles 0-3 only need chunk-0
            # yT columns (all pairs), so they can slot into the last
            # pair's chunk-1 emission; t-tiles 4-7 run at the tail.
            op_units = [(it, off, w) for it in range(8)
                        for off, w in ((0, 512), (512, 256))]
            fill_iter = iter(op_units[:8])

            def filler():
                u = next(fill_iter, None)
                if u is not None:
                    emit_outproj_unit(*u)

            for hp in range(NPAIR):
                qk_t = qk_next
                last = hp == NPAIR - 1
                if not last:
                    wt_next = emit_qkproj_dma(hp + 1)
                y0 = emit_attn_core(hp, qk_t, 0)
                if not last:
                    qk_next = emit_qkproj(hp + 1, wt_next)

                    # norm of chunk 0 is emitted inside chunk 1 (g=1):
                    # if it were emitted first, its DVE ops would jump
                    # ahead of the parked qk copies in the DVE wait
                    # queue and delay the PSUM recycle.
                    def norm0(hp=hp, y0=y0):
                        emit_norm(hp, 0, y0)

                    y1 = emit_attn_core(hp, qk_t, 1, pre_g={1: norm0})
                else:
                    emit_norm(hp, 0, y0)
                    y1 = emit_attn_core(hp, qk_t, 1, filler=filler)
                emit_norm(hp, 1, y1, stage_y=not last)

            for u in fill_iter:
                emit_outproj_unit(*u, on_scalar=True)
            for u in op_units[8:]:
                emit_outproj_unit(*u, on_scalar=True)

    nc.compile()
    return nc


def _get_program(bias_attn, bias_proj):
    key = (bias_attn, bias_proj)
    if key not in _cache:
        _cache[key] = _build_program(bias_attn, bias_proj)
    return _cache[key]


def _prep_inputs(x, w_attn, b_attn, w_proj, b_proj):
    x = np.asarray(x, dtype=np.float32)
    w_attn = np.asarray(w_attn, dtype=np.float32)
    b_attn = np.asarray(b_attn, dtype=np.float32)
    w_proj = np.asarray(w_proj, dtype=np.float32)
    b_proj = np.asarray(b_proj, dtype=np.float32)
    bias_attn = bool(np.any(b_attn))
    bias_proj = bool(np.any(b_proj))
    import ml_dtypes
    bf = ml_dtypes.bfloat16

    # Pre-tile to [128, free-contiguous] layouts (see _build_program).
    # wqk[p, hp, k, u, m] = w_attn[k*128+p, u*C + hp*128 + m]
    wqk = np.ascontiguousarray(
        w_attn[:, :2 * C].reshape(KT, 128, 2, NPAIR, 128)
        .transpose(1, 3, 0, 2, 4).reshape(128, -1)).astype(bf)
    # wv[p, k, m] = w_attn[k*128+p, 2C + m]
    wv = np.ascontiguousarray(
        w_attn[:, 2 * C:].reshape(KT, 128, C)
        .transpose(1, 0, 2).reshape(128, -1)).astype(bf)
    # wp[p, k, m] = w_proj[k*128+p, m]
    wpb = np.ascontiguousarray(
        w_proj.reshape(KT, 128, C).transpose(1, 0, 2).reshape(128, -1)
    ).astype(bf)
    in_maps = []
    for b in range(NCORES):
        # xT[p, k, t] = x[b, t, k*128+p]
        xt = np.ascontiguousarray(
            x[b].T.reshape(KT, 128, T).transpose(1, 0, 2).reshape(128, -1)
        ).astype(bf)
        m = {
            "xT": xt,
            "wqk": wqk,
            "wv": wv,
            "wp": wpb,
        }
        if bias_attn:
            m["bqk"] = np.ascontiguousarray(b_attn[:2 * C])
            m["bv"] = np.ascontiguousarray(b_attn[2 * C:])
        if bias_proj:
            m["bp"] = b_proj
        in_maps.append(m)
    return in_maps, bias_attn, bias_proj


def run(x, w_attn, b_attn, w_proj, b_proj, trace=False, tmpdir=None):
    in_maps, bias_attn, bias_proj = _prep_inputs(
        x, w_attn, b_attn, w_proj, b_proj)
    nc = _get_program(bias_attn, bias_proj)
    res = run_bass_kernel_spmd(nc, in_maps, list(range(NCORES)),
                               trace=trace, tmpdir=tmpdir)
    # out[p, it, m] -> [T, C]
    out = np.stack(
        [np.ascontiguousarray(
            res.results[i]["out"].reshape(128, 8, C).transpose(1, 0, 2)
            .reshape(T, C)) for i in range(NCORES)], axis=0)
    return out.astype(np.float32), res


def kernel(x, w_attn, b_attn, w_proj, b_proj):
    out, _ = run(x, w_attn, b_attn, w_proj, b_proj)
    return out


# revision 41
# speedup vs baseline: 1.0328x; 1.0257x over previous
"""Causal self-attention Trainium2 Bass kernel.

Shapes (hardcoded): B=8, T=1024, C=768, NH=12, HS=64.
Sharding: data-parallel over batch — core b computes batch element b.

Per-core dataflow (all matmuls bf16 with fp32 PSUM accumulation):
  - All DRAM operands are pre-tiled on the host so every DMA is 128
    partitions x long contiguous runs; xT and wv are split into per-k
    DMAs so the first projections start as soon as their tiles land.
  - qkT  [2C, T] channel-major  = w_qk.T-tiles (stationary) x xT (moving).
    Pair hp+1's QK projection is emitted between chunk 0 and chunk 1 of
    pair hp. The PSUM->SBUF copies ride the Vector queue, emitted
    between norm stage 1 and stage 2 of chunk 0 so they are not stuck
    behind the full normalize chain.
  - v    token-major [T, C], assembled into v_aug [jt, head, 65] with a
    ones column so the PV matmul also emits softmax row-sums for free
  - S^T  [j, i] blocks per head: lhsT = kT j-tile (K=64), rhs = qT i-cols.
    Causality via block skipping plus an additive -1920 lower-triangular
    constant accumulated into diagonal blocks by a bf16 matmul (id.T @
    mtri); exp(0.125*(S-1920)) underflows to exact 0.
  - exp via ScalarE activation (scale=1/8) PSUM->SBUF into bf16 P^T.
    PV of group g-1 is interleaved between S^T groups.
  - y^T [65, i] = v_aug.T x P^T accumulated over j-tiles in PSUM; row 64
    is the softmax denominator. Norm stage 1 (DVE): yst copy frees the
    PSUM slot, sums-row copy + reciprocal_approx_fast; stage 2: gpsimd
    partition_broadcast + DVE multiply into bf16 yT [C, T]. The last
    chunk skips the yst staging (multiplies straight out of PSUM) to
    shorten the tail dependency chain.
  - out [T, C] = yT-tiles (stationary) x w_proj (moving), DVE copy to
    bf16 [128, it, C] tiles, one DMA per t-tile (host un-tiles and
    casts back to f32). The 16 projection units are interleaved into
    the last pair's chunk-1 emission and the tail.
"""

import numpy as np

import concourse.bass as bass
import concourse.mybir as mybir
import concourse.tile as tile
from concourse import bacc
from concourse.bass_utils import run_bass_kernel_spmd

B, T, C = 8, 1024, 768
NH, HS = 12, 64
NCORES = 8
KT = C // 128            # 6 contraction tiles
NPAIR = NH // 2          # 6 head pairs; head-pair hp covers heads 2hp, 2hp+1
F32 = mybir.dt.float32
BF16 = mybir.dt.bfloat16

_cache = {}


def _build_program(bias_attn: bool, bias_proj: bool):
    nc = bacc.Bacc("TRN2", target_bir_lowering=False, debug=False,
                   num_devices=NCORES)

    # Pre-tiled DRAM layouts (see _prep_inputs): every tensor is
    # [128, ...] with the full free dim contiguous per partition.
    xT = nc.dram_tensor("xT", [128, KT * T], BF16, kind="ExternalInput")
    wqk = nc.dram_tensor("wqk", [128, NPAIR * KT * 2 * 128], BF16,
                         kind="ExternalInput")
    wv = nc.dram_tensor("wv", [128, KT * C], BF16, kind="ExternalInput")
    wp = nc.dram_tensor("wp", [128, KT * C], BF16, kind="ExternalInput")
    if bias_attn:
        bqk_d = nc.dram_tensor("bqk", [2 * C], F32, kind="ExternalInput")
        bv_d = nc.dram_tensor("bv", [C], F32, kind="ExternalInput")
    if bias_proj:
        bp_d = nc.dram_tensor("bp", [C], F32, kind="ExternalInput")
    out = nc.dram_tensor("out", [128, 8 * C], BF16, kind="ExternalOutput")
    out_r = out.ap().rearrange("p (i m) -> p i m", m=C)

    # Additive causal mask for diagonal S^T blocks, applied as a bf16
    # accumulation matmul (identity.T @ mtri adds mtri to the PSUM tile):
    # -1920 pre-scale = -240 post-scale -> exp underflows to exactly 0.
    import ml_dtypes
    mtri_np = (np.tril(np.ones((128, 128)), k=-1) * -1920.0).astype(
        ml_dtypes.bfloat16)
    # two copies side by side: one 256-col matmul masks both diagonal
    # blocks of an S^T group (out AP strides across the two u-strips)
    mtri2_d = nc.inline_tensor(
        np.concatenate([mtri_np, mtri_np], axis=1), "mtri2")
    id_np = np.eye(128, dtype=ml_dtypes.bfloat16)
    id_d = nc.inline_tensor(id_np, "id128")

    xT_r = xT.ap().rearrange("p (k t) -> p k t", t=T)
    wqk_r = wqk.ap().rearrange("p (hp k u m) -> p hp k u m", k=KT, u=2, m=128)
    wv_r = wv.ap().rearrange("p (k m) -> p k m", m=C)
    wp_r = wp.ap().rearrange("p (k m) -> p k m", m=C)

    with tile.TileContext(nc) as tc:
        with (
            tc.tile_pool(name="xpool", bufs=1) as xpool,
            tc.tile_pool(name="cpool", bufs=1) as cpool,
            tc.tile_pool(name="wvpool", bufs=1) as wvpool,
            tc.tile_pool(name="vpool", bufs=1) as vpool,
            tc.tile_pool(name="wqkpool", bufs=3) as wqkpool,
            tc.tile_pool(name="qkpool", bufs=3) as qkpool,
            tc.tile_pool(name="ptpool", bufs=6) as ptpool,
            tc.tile_pool(name="ytpool", bufs=1) as ytpool,
            tc.tile_pool(name="wppool", bufs=1) as wppool,
            tc.tile_pool(name="opool", bufs=3) as opool,
            tc.tile_pool(name="smpool", bufs=6) as smpool,
            tc.tile_pool(name="psA", bufs=3, space="PSUM") as psA,
            tc.tile_pool(name="psB", bufs=2, space="PSUM") as psB,
        ):
            # ---- first-pair weights then x (per k-tile): the first QK
            # matmul starts as soon as wt_0 + xT k-tile 0 land. Every
            # SP DMA issue costs ~0.6us serially, so order strictly by
            # when the data is needed. ----
            def emit_qkproj_dma(hp):
                wt = wqkpool.tile([128, KT, 2, 128], BF16, tag="wqk",
                                  name=f"wt_{hp}")
                nc.sync.dma_start(wt[:], wqk_r[:, hp])
                return wt

            wt_next = emit_qkproj_dma(0)

            xT_s = xpool.tile([128, KT, T], BF16, tag="xT")
            for k in range(KT):
                nc.sync.dma_start(xT_s[:, k, :], xT_r[:, k, :])

            mtri2_s = cpool.tile([128, 256], BF16, tag="mtri2")
            nc.sync.dma_start(mtri2_s[:], mtri2_d.ap())
            id_s = cpool.tile([128, 128], BF16, tag="id128")
            nc.sync.dma_start(id_s[:], id_d.ap())

            # Chain wv behind xT (dummy copy creates the dependency) so
            # its transfer does not steal HBM bandwidth from the
            # critical-path xT tiles; likewise wpt behind wv.
            wv_s = wvpool.tile([128, KT, C], BF16, tag="wv")
            nc.vector.tensor_copy(wv_s[0:1, KT - 1, C - 1:C],
                                  xT_s[0:1, KT - 1, T - 1:T])
            for k in range(KT):
                nc.sync.dma_start(wv_s[:, k, :], wv_r[:, k, :])

            # ---- bias staging ----
            if bias_attn:
                bqk_s = cpool.tile([128, 12], F32, tag="bqk")
                nc.sync.dma_start(bqk_s[:], bqk_d.ap().rearrange("(m p) -> p m", p=128))
                bv_row = cpool.tile([1, C], F32, tag="bvrow")
                nc.sync.dma_start(bv_row[:], bv_d.ap().rearrange("c -> 1 c"))
                bv_bc = cpool.tile([128, C], F32, tag="bvbc")
                nc.gpsimd.partition_broadcast(bv_bc[:], bv_row[:])
            if bias_proj:
                bp_row = cpool.tile([1, C], F32, tag="bprow")
                nc.sync.dma_start(bp_row[:], bp_d.ap().rearrange("c -> 1 c"))
                bp_bc = cpool.tile([128, C], F32, tag="bpbc")
                nc.gpsimd.partition_broadcast(bp_bc[:], bp_row[:])



            def emit_qkproj(hp, wt):
                qk_t = qkpool.tile([128, 2, T], BF16, tag="qk",
                                   name=f"qk_{hp}")
                for part in range(2):  # 0 = q m-tile hp, 1 = k m-tile hp
                    ps = psA.tile([128, 1024], F32, tag="big",
                                  name=f"qkps_{hp}_{part}")
                    # Per-half PSUM->SBUF copies on DVE directly behind
                    # each half's matmuls: by the time the in-order DVE
                    # queue reaches a copy its dep is satisfied, so no
                    # wait-queue parking and the PSUM slot frees ~one
                    # half earlier. (Matmul output must stay within one
                    # 2KB PSUM bank: 512 fp32 cols max.)
                    for nch in range(2):
                        sl = slice(nch * 512, (nch + 1) * 512)
                        for k in range(KT):
                            nc.tensor.matmul(
                                ps[:, sl],
                                wt[:, k, part, :],
                                xT_s[:, k, sl],
                                start=(k == 0), stop=(k == KT - 1),
                            )
                        if bias_attn:
                            nc.vector.tensor_add(
                                qk_t[:, part, sl], ps[:, sl],
                                bqk_s[:, part * 6 + hp:part * 6 + hp + 1])
                        else:
                            nc.vector.tensor_copy(qk_t[:, part, sl],
                                                  ps[:, sl])
                return qk_t

            qk_next = emit_qkproj(0, wt_next)

            # ---- V: token-major, assembled as v_aug[jt, head, 128]
            # with cols 64-127 all-ones: the PV matmul then emits 64
            # identical softmax-denominator rows into PSUM partitions
            # 64-127 for free (matmul time is independent of output
            # partition count), so the normalize chain runs 64-wide
            # with no partition_broadcast.  ----
            v_aug = vpool.tile([128, 8, NH, 128], BF16, tag="vaug")
            nc.vector.memset(v_aug[:, :, :, HS:128], 1.0)
            for jt in range(8):
                ps = psA.tile([128, 1024], F32, tag="big")
                for off, w in ((0, 512), (512, 256)):
                    for k in range(KT):
                        nc.tensor.matmul(
                            ps[:, off:off + w],
                            xT_s[:, k, jt * 128:(jt + 1) * 128],
                            wv_s[:, k, off:off + w],
                            start=(k == 0), stop=(k == KT - 1),
                        )
                dst = v_aug[:, jt, :, 0:HS]
                src = ps[:, 0:C].rearrange("p (h d) -> p h d", d=HS)
                if bias_attn:
                    nc.vector.tensor_add(
                        dst, src, bv_bc[:].rearrange("p (h d) -> p h d", d=HS))
                else:
                    nc.vector.tensor_copy(dst, src)

            # ---- yT accumulator (written during attention) ----
            yT_s = ytpool.tile([128, KT, T], BF16, tag="yT")

            # prefetch projection weights (consumed only at the tail);
            # chained behind wv to keep early HBM bandwidth for x
            wpt = wppool.tile([128, KT, C], BF16, tag="wp")
            nc.vector.tensor_copy(wpt[0:1, KT - 1, C - 1:C],
                                  wv_s[0:1, KT - 1, C - 1:C])
            nc.sync.dma_start(wpt[:], wp_r)

            # ---- output projection, one t-tile unit at a time so
            # units can be interleaved into the last pair ----

            def emit_outproj_unit(it, on_scalar=False):
                ps = psA.tile([128, 1024], F32, tag="big",
                              name=f"ops_{it}")
                for off, w in ((0, 512), (512, 256)):
                    for k in range(KT):
                        nc.tensor.matmul(
                            ps[:, off:off + w],
                            yT_s[:, k, it * 128:(it + 1) * 128],
                            wpt[:, k, off:off + w],
                            start=(k == 0), stop=(k == KT - 1),
                        )
                ot = opool.tile([128, C], BF16, tag="ot",
                                name=f"ot_{it}")
                if bias_proj:
                    nc.vector.tensor_add(ot[:], ps[:, 0:C], bp_bc[:])
                elif on_scalar:
                    # tail units: ACT is idle after the last exp
                    nc.scalar.copy(ot[:], ps[:, 0:C])
                else:
                    nc.vector.tensor_copy(ot[:], ps[:, 0:C])
                nc.sync.dma_start(out_r[:, it, :], ot[:])

            # ---- attention core: S^T, exp, PV for one (pair, chunk) ----
            def emit_attn_core(hp, qk_t, c, filler=None, pre_g=None):
                njt = 4 * (c + 1)
                pts = [[ptpool.tile([128, 4, 512], BF16, tag="pt",
                                    name=f"pt_{hp}_{hl}_{c}_{i}")
                        for i in range(njt // 4)] for hl in range(2)]

                def pt_ap(hl, jt):
                    return pts[hl][jt // 4][:, jt % 4, :]

                y_pss = [psB.tile([128, 512], F32, tag="y",
                                  name=f"yps_{hp}_{hl}_{c}")
                         for hl in range(2)]

                def emit_pv(g):
                    # PV for the two j-tiles of group g (both heads)
                    for u in range(2):
                        jt = 2 * g + u
                        lo = max(0, (jt - 4 * c) * 128)
                        for hl in range(2):
                            nc.tensor.matmul(
                                y_pss[hl][:, lo:512],
                                v_aug[:, jt, 2 * hp + hl, :],
                                pt_ap(hl, jt)[:, lo:512],
                                start=(jt == 0),
                                stop=(jt == njt - 1),
                                skip_group_check=(jt > 0),
                            )

                # Software-pipelined emission: PV of group g-2 is
                # interleaved between S^T groups so the in-order PE
                # queue always has work that does not wait on the
                # ScalarE exp (which drains the S^T PSUM slots) and the
                # first PV lands after the previous chunk's norm frees
                # the y PSUM slot.
                LAG = 2
                for g in range(njt // 2):
                    if pre_g is not None and g in pre_g:
                        pre_g[g]()
                    sts = [psA.tile([128, 1024], F32, tag="big",
                                    name=f"st_{hp}_{hl}_{c}_{g}")
                           for hl in range(2)]
                    diag_g = 2 * g >= 4 * c
                    logp = max(0, (2 * g - 4 * c)) * 128
                    # heads sequential (not interleaved): head hl's exp
                    # is emitted before head hl+1's S^T so it starts ~1us
                    # earlier and its PSUM slot recycles sooner
                    for hl in range(2):
                        base = 64 * hl
                        for u in range(2):
                            jt = 2 * g + u
                            lo = max(0, (jt - 4 * c) * 128)
                            nc.tensor.matmul(
                                sts[hl][:, u * 512 + lo:(u + 1) * 512],
                                qk_t[base:base + 64, 1,
                                     jt * 128:(jt + 1) * 128],
                                qk_t[base:base + 64, 0,
                                     c * 512 + lo:(c + 1) * 512],
                                start=True, stop=not diag_g,
                            )
                        if diag_g:
                            # accumulate -1920 on the j>i triangles of
                            # both diagonal blocks with one 256-col
                            # matmul: the u0 block sits at sub-block b0,
                            # the u1 block 5 sub-blocks later
                            b0 = 2 * g - 4 * c
                            dst = sts[hl][:].rearrange(
                                "p (x n) -> p x n", n=128)[:, b0:b0 + 6:5, :]
                            nc.tensor.matmul(
                                dst, id_s[:],
                                mtri2_s[:].rearrange("p (u n) -> p u n",
                                                     n=128),
                                start=False, stop=True,
                                skip_group_check=True,
                            )
                        # exp only the live columns [lo(2g):512] of both
                        # strips — columns left of the first j-tile's lo
                        # are never read by PV
                        nc.scalar.activation(
                            pts[hl][g // 2][:, (g % 2) * 2:(g % 2) * 2 + 2,
                                            logp:512],
                            sts[hl][:].rearrange("p (a n) -> p a n",
                                                 n=512)[:, :, logp:512],
                            mybir.ActivationFunctionType.Exp,
                            scale=0.125,
                        )
                    if g >= LAG:
                        emit_pv(g - LAG)
                    if filler is not None:
                        filler()
                for gg in range(max(0, njt // 2 - LAG), njt // 2):
                    emit_pv(gg)
                return y_pss

            # ---- normalize: yT[h rows, c cols] = y / sums ----
            # Stage 1 (DVE): yst copy frees the y PSUM slot; sums-row
            # copy from PSUM (shifts partition 64 -> 0; the SBUF->SBUF
            # shifted copy reads the wrong lane) and reciprocal in
            # place (custom-DVE needs base partition 0).
            # normalize: yT = y / sums, all on DVE, 64 lanes wide (the
            # PV matmul put 64 copies of the sums in PSUM rows 64-127).
            def emit_norm(hp, c, y_pss, stage_y=True):
                ysts, srows = [], []
                for hl in range(2):
                    if stage_y:
                        yst = smpool.tile([64, 512], F32, tag="yst",
                                          name=f"yst_{hp}_{hl}_{c}")
                        nc.vector.tensor_copy(yst[:], y_pss[hl][0:HS, :])
                    else:
                        yst = None
                    ysts.append(yst)
                for hl in range(2):
                    srow = smpool.tile([64, 512], F32, tag="srow",
                                       name=f"srow_{hp}_{hl}_{c}")
                    nc.vector.tensor_copy(srow[:], y_pss[hl][HS:128, :])
                    nc.vector.reciprocal_approx_fast(srow[:], srow[:])
                    srows.append(srow)
                for hl in range(2):
                    src = (ysts[hl][:] if ysts[hl] is not None
                           else y_pss[hl][0:HS, :])
                    nc.vector.tensor_mul(
                        yT_s[64 * hl:64 * hl + 64, hp,
                             c * 512:(c + 1) * 512],
                        src, srows[hl][:])

            # Output-projection units for t-tiles 0-3 only need chunk-0
            # yT columns (all pairs), so they can slot into the last
            # pair's chunk-1 emission; t-tiles 4-7 run at the tail.
            fill_iter = iter(range(4))

            def filler():
                u = next(fill_iter, None)
                if u is not None:
                    emit_outproj_unit(u)

            for hp in range(NPAIR):
                qk_t = qk_next
                last = hp == NPAIR - 1
                if not last:
                    wt_next = emit_qkproj_dma(hp + 1)
                y0 = emit_attn_core(hp, qk_t, 0)
                if not last:
                    qk_next = emit_qkproj(hp + 1, wt_next)

                    # norm of chunk 0 is emitted inside chunk 1 (g=1):
                    # if it were emitted first, its DVE ops would jump
                    # ahead of the parked qk copies in the DVE wait
                    # queue and delay the PSUM recycle.
                    def norm0(hp=hp, y0=y0):
                        emit_norm(hp, 0, y0)

                    y1 = emit_attn_core(hp, qk_t, 1, pre_g={1: norm0})
                else:
                    emit_norm(hp, 0, y0)
                    y1 = emit_attn_core(hp, qk_t, 1, filler=filler)
                emit_norm(hp, 1, y1, stage_y=not last)

            for u in fill_iter:
                emit_outproj_unit(u, on_scalar=True)
            for u in range(4, 8):
                emit_outproj_unit(u, on_scalar=True)

    nc.compile()
    return nc


def _get_program(bias_attn, bias_proj):
    key = (bias_attn, bias_proj)
    if key not in _cache:
        _cache[key] = _build_program(bias_attn, bias_proj)
    return _cache[key]


def _prep_inputs(x, w_attn, b_attn, w_proj, b_proj):
    x = np.asarray(x, dtype=np.float32)
    w_attn = np.asarray(w_attn, dtype=np.float32)
    b_attn = np.asarray(b_attn, dtype=np.float32)
    w_proj = np.asarray(w_proj, dtype=np.float32)
    b_proj = np.asarray(b_proj, dtype=np.float32)
    bias_attn = bool(np.any(b_attn))
    bias_proj = bool(np.any(b_proj))
    import ml_dtypes
    bf = ml_dtypes.bfloat16

    # Pre-tile to [128, free-contiguous] layouts (see _build_program).
    # wqk[p, hp, k, u, m] = w_attn[k*128+p, u*C + hp*128 + m]
    wqk = np.ascontiguousarray(
        w_attn[:, :2 * C].reshape(KT, 128, 2, NPAIR, 128)
        .transpose(1, 3, 0, 2, 4).reshape(128, -1)).astype(bf)
    # wv[p, k, m] = w_attn[k*128+p, 2C + m]
    wv = np.ascontiguousarray(
        w_attn[:, 2 * C:].reshape(KT, 128, C)
        .transpose(1, 0, 2).reshape(128, -1)).astype(bf)
    # wp[p, k, m] = w_proj[k*128+p, m]
    wpb = np.ascontiguousarray(
        w_proj.reshape(KT, 128, C).transpose(1, 0, 2).reshape(128, -1)
    ).astype(bf)
    in_maps = []
    for b in range(NCORES):
        # xT[p, k, t] = x[b, t, k*128+p]
        xt = np.ascontiguousarray(
            x[b].T.reshape(KT, 128, T).transpose(1, 0, 2).reshape(128, -1)
        ).astype(bf)
        m = {
            "xT": xt,
            "wqk": wqk,
            "wv": wv,
            "wp": wpb,
        }
        if bias_attn:
            m["bqk"] = np.ascontiguousarray(b_attn[:2 * C])
            m["bv"] = np.ascontiguousarray(b_attn[2 * C:])
        if bias_proj:
            m["bp"] = b_proj
        in_maps.append(m)
    return in_maps, bias_attn, bias_proj


def run(x, w_attn, b_attn, w_proj, b_proj, trace=False, tmpdir=None):
    in_maps, bias_attn, bias_proj = _prep_inputs(
        x, w_attn, b_attn, w_proj, b_proj)
    nc = _get_program(bias_attn, bias_proj)
    res = run_bass_kernel_spmd(nc, in_maps, list(range(NCORES)),
                               trace=trace, tmpdir=tmpdir)
    # out[p, it, m] -> [T, C]
    out = np.stack(
        [np.ascontiguousarray(
            res.results[i]["out"].reshape(128, 8, C).transpose(1, 0, 2)
            .reshape(T, C)) for i in range(NCORES)], axis=0)
    return out.astype(np.float32), res


def kernel(x, w_attn, b_attn, w_proj, b_proj):
    out, _ = run(x, w_attn, b_attn, w_proj, b_proj)
    return out


# revision 44
# speedup vs baseline: 1.0532x; 1.0198x over previous
"""Causal self-attention Trainium2 Bass kernel.

Shapes (hardcoded): B=8, T=1024, C=768, NH=12, HS=64.
Sharding: data-parallel over batch — core b computes batch element b.

Per-core dataflow (all matmuls bf16 with fp32 PSUM accumulation):
  - All DRAM operands are pre-tiled on the host so every DMA is 128
    partitions x long contiguous runs; xT and wv are split into per-k
    DMAs so the first projections start as soon as their tiles land.
  - qkT  [2C, T] channel-major  = w_qk.T-tiles (stationary) x xT (moving).
    Pair hp+1's QK projection is emitted between chunk 0 and chunk 1 of
    pair hp. The PSUM->SBUF copies ride the Vector queue, emitted
    between norm stage 1 and stage 2 of chunk 0 so they are not stuck
    behind the full normalize chain.
  - v    token-major [T, C], assembled into v_aug [jt, head, 65] with a
    ones column so the PV matmul also emits softmax row-sums for free
  - S^T  [j, i] blocks per head: lhsT = kT j-tile (K=64), rhs = qT i-cols.
    Causality via block skipping plus an additive -1920 lower-triangular
    constant accumulated into diagonal blocks by a bf16 matmul (id.T @
    mtri); exp(0.125*(S-1920)) underflows to exact 0.
  - exp via ScalarE activation (scale=1/8) PSUM->SBUF into bf16 P^T.
    PV of group g-1 is interleaved between S^T groups.
  - y^T [65, i] = v_aug.T x P^T accumulated over j-tiles in PSUM; row 64
    is the softmax denominator. Norm stage 1 (DVE): yst copy frees the
    PSUM slot, sums-row copy + reciprocal_approx_fast; stage 2: gpsimd
    partition_broadcast + DVE multiply into bf16 yT [C, T]. The last
    chunk skips the yst staging (multiplies straight out of PSUM) to
    shorten the tail dependency chain.
  - out [T, C] = yT-tiles (stationary) x w_proj (moving), DVE copy to
    bf16 [128, it, C] tiles, one DMA per t-tile (host un-tiles and
    casts back to f32). The 16 projection units are interleaved into
    the last pair's chunk-1 emission and the tail.
"""

import numpy as np

import concourse.bass as bass
import concourse.mybir as mybir
import concourse.tile as tile
from concourse import bacc
from concourse.bass_utils import run_bass_kernel_spmd

B, T, C = 8, 1024, 768
NH, HS = 12, 64
NCORES = 8
KT = C // 128            # 6 contraction tiles
NPAIR = NH // 2          # 6 head pairs; head-pair hp covers heads 2hp, 2hp+1
F32 = mybir.dt.float32
BF16 = mybir.dt.bfloat16

_cache = {}


def _build_program(bias_attn: bool, bias_proj: bool):
    nc = bacc.Bacc("TRN2", target_bir_lowering=False, debug=False,
                   num_devices=NCORES)

    # Pre-tiled DRAM layouts (see _prep_inputs): every tensor is
    # [128, ...] with the full free dim contiguous per partition.
    xT = nc.dram_tensor("xT", [128, KT * T], BF16, kind="ExternalInput")
    wqk = nc.dram_tensor("wqk", [128, NPAIR * KT * 2 * 128], BF16,
                         kind="ExternalInput")
    wv = nc.dram_tensor("wv", [128, KT * C], BF16, kind="ExternalInput")
    wp = nc.dram_tensor("wp", [128, KT * C], BF16, kind="ExternalInput")
    if bias_attn:
        bqk_d = nc.dram_tensor("bqk", [2 * C], F32, kind="ExternalInput")
        bv_d = nc.dram_tensor("bv", [C], F32, kind="ExternalInput")
    if bias_proj:
        bp_d = nc.dram_tensor("bp", [C], F32, kind="ExternalInput")
    out = nc.dram_tensor("out", [128, 8 * C], BF16, kind="ExternalOutput")
    out_r = out.ap().rearrange("p (i m) -> p i m", m=C)

    # Additive causal mask for diagonal S^T blocks, applied as a bf16
    # accumulation matmul (identity.T @ mtri adds mtri to the PSUM tile):
    # -1920 pre-scale = -240 post-scale -> exp underflows to exactly 0.
    import ml_dtypes
    mtri_np = (np.tril(np.ones((128, 128)), k=-1) * -1920.0).astype(
        ml_dtypes.bfloat16)
    # two copies side by side: one 256-col matmul masks both diagonal
    # blocks of an S^T group (out AP strides across the two u-strips)
    mtri2_d = nc.inline_tensor(
        np.concatenate([mtri_np, mtri_np], axis=1), "mtri2")
    id_np = np.eye(128, dtype=ml_dtypes.bfloat16)
    id_d = nc.inline_tensor(id_np, "id128")

    xT_r = xT.ap().rearrange("p (k t) -> p k t", t=T)
    wqk_r = wqk.ap().rearrange("p (hp k u m) -> p hp k u m", k=KT, u=2, m=128)
    wv_r = wv.ap().rearrange("p (k m) -> p k m", m=C)
    wp_r = wp.ap().rearrange("p (k m) -> p k m", m=C)

    with tile.TileContext(nc) as tc:
        with (
            tc.tile_pool(name="xpool", bufs=1) as xpool,
            tc.tile_pool(name="cpool", bufs=1) as cpool,
            tc.tile_pool(name="wvpool", bufs=1) as wvpool,
            tc.tile_pool(name="vpool", bufs=1) as vpool,
            tc.tile_pool(name="wqkpool", bufs=3) as wqkpool,
            tc.tile_pool(name="qkpool", bufs=3) as qkpool,
            tc.tile_pool(name="ptpool", bufs=6) as ptpool,
            tc.tile_pool(name="ytpool", bufs=1) as ytpool,
            tc.tile_pool(name="wppool", bufs=1) as wppool,
            tc.tile_pool(name="opool", bufs=3) as opool,
            tc.tile_pool(name="smpool", bufs=6) as smpool,
            tc.tile_pool(name="psA", bufs=3, space="PSUM") as psA,
            tc.tile_pool(name="psB", bufs=2, space="PSUM") as psB,
        ):
            # ---- first-pair weights then x (per k-tile): the first QK
            # matmul starts as soon as wt_0 + xT k-tile 0 land. Every
            # SP DMA issue costs ~0.6us serially, so order strictly by
            # when the data is needed. ----
            def emit_qkproj_dma(hp):
                wt = wqkpool.tile([128, KT, 2, 128], BF16, tag="wqk",
                                  name=f"wt_{hp}")
                nc.sync.dma_start(wt[:], wqk_r[:, hp])
                return wt

            wt_next = emit_qkproj_dma(0)

            xT_s = xpool.tile([128, KT, T], BF16, tag="xT")
            for k in range(KT):
                nc.sync.dma_start(xT_s[:, k, :], xT_r[:, k, :])

            # Persistent qk buffers [128, 3, T]: dim1 0 = q (both heads
            # stacked 64+64), 1 = k of head 0 zero-padded to 128 rows,
            # 2 = k of head 1 zero-padded. The padding lets S^T run
            # with 128-partition operands (full stream rate + FWL) —
            # the other head's q rows multiply by zero weights. The
            # zero halves are memset once; buffers rotate across pairs.
            qk2_bufs = [qkpool.tile([128, 3, T], BF16, tag="qk",
                                    name=f"qk2_{i}") for i in range(3)]
            for b in qk2_bufs:
                nc.vector.memset(b[64:128, 1, :], 0.0)
                nc.vector.memset(b[0:64, 2, :], 0.0)

            mtri2_s = cpool.tile([128, 256], BF16, tag="mtri2")
            nc.sync.dma_start(mtri2_s[:], mtri2_d.ap())
            id_s = cpool.tile([128, 128], BF16, tag="id128")
            nc.sync.dma_start(id_s[:], id_d.ap())

            # Chain wv behind xT (dummy copy creates the dependency) so
            # its transfer does not steal HBM bandwidth from the
            # critical-path xT tiles; likewise wpt behind wv.
            wv_s = wvpool.tile([128, KT, C], BF16, tag="wv")
            nc.vector.tensor_copy(wv_s[0:1, KT - 1, C - 1:C],
                                  xT_s[0:1, KT - 1, T - 1:T])
            for k in range(KT):
                nc.sync.dma_start(wv_s[:, k, :], wv_r[:, k, :])

            # ---- bias staging ----
            if bias_attn:
                bqk_s = cpool.tile([128, 12], F32, tag="bqk")
                nc.sync.dma_start(bqk_s[:], bqk_d.ap().rearrange("(m p) -> p m", p=128))
                bv_row = cpool.tile([1, C], F32, tag="bvrow")
                nc.sync.dma_start(bv_row[:], bv_d.ap().rearrange("c -> 1 c"))
                bv_bc = cpool.tile([128, C], F32, tag="bvbc")
                nc.gpsimd.partition_broadcast(bv_bc[:], bv_row[:])
            if bias_proj:
                bp_row = cpool.tile([1, C], F32, tag="bprow")
                nc.sync.dma_start(bp_row[:], bp_d.ap().rearrange("c -> 1 c"))
                bp_bc = cpool.tile([128, C], F32, tag="bpbc")
                nc.gpsimd.partition_broadcast(bp_bc[:], bp_row[:])



            def emit_qkproj(hp, wt):
                qk_t = qk2_bufs[hp % 3]
                for part in range(2):  # 0 = q m-tile hp, 1 = k m-tile hp
                    ps = psA.tile([128, 1024], F32, tag="big",
                                  name=f"qkps_{hp}_{part}")
                    # Per-half PSUM->SBUF copies on DVE directly behind
                    # each half's matmuls: by the time the in-order DVE
                    # queue reaches a copy its dep is satisfied, so no
                    # wait-queue parking and the PSUM slot frees ~one
                    # half earlier. (Matmul output must stay within one
                    # 2KB PSUM bank: 512 fp32 cols max.)
                    for nch in range(2):
                        sl = slice(nch * 512, (nch + 1) * 512)
                        for k in range(KT):
                            nc.tensor.matmul(
                                ps[:, sl],
                                wt[:, k, part, :],
                                xT_s[:, k, sl],
                                start=(k == 0), stop=(k == KT - 1),
                            )
                        if part == 0:
                            if bias_attn:
                                nc.vector.tensor_add(
                                    qk_t[:, 0, sl], ps[:, sl],
                                    bqk_s[:, hp:hp + 1])
                            else:
                                nc.vector.tensor_copy(qk_t[:, 0, sl],
                                                      ps[:, sl])
                        else:
                            # k rows split per head into the padded slots
                            if bias_attn:
                                nc.vector.tensor_add(
                                    qk_t[0:64, 1, sl], ps[0:64, sl],
                                    bqk_s[0:64, 6 + hp:7 + hp])
                                nc.vector.tensor_add(
                                    qk_t[64:128, 2, sl], ps[64:128, sl],
                                    bqk_s[64:128, 6 + hp:7 + hp])
                            else:
                                nc.vector.tensor_copy(qk_t[0:64, 1, sl],
                                                      ps[0:64, sl])
                                nc.vector.tensor_copy(qk_t[64:128, 2, sl],
                                                      ps[64:128, sl])
                return qk_t

            qk_next = emit_qkproj(0, wt_next)

            # ---- V: token-major, assembled as v_aug[jt, head, 128]
            # with cols 64-127 all-ones: the PV matmul then emits 64
            # identical softmax-denominator rows into PSUM partitions
            # 64-127 for free (matmul time is independent of output
            # partition count), so the normalize chain runs 64-wide
            # with no partition_broadcast.  ----
            v_aug = vpool.tile([128, 8, NH, 128], BF16, tag="vaug")
            nc.vector.memset(v_aug[:, :, :, HS:128], 1.0)
            for jt in range(8):
                ps = psA.tile([128, 1024], F32, tag="big")
                for off, w in ((0, 512), (512, 256)):
                    for k in range(KT):
                        nc.tensor.matmul(
                            ps[:, off:off + w],
                            xT_s[:, k, jt * 128:(jt + 1) * 128],
                            wv_s[:, k, off:off + w],
                            start=(k == 0), stop=(k == KT - 1),
                        )
                dst = v_aug[:, jt, :, 0:HS]
                src = ps[:, 0:C].rearrange("p (h d) -> p h d", d=HS)
                if bias_attn:
                    nc.vector.tensor_add(
                        dst, src, bv_bc[:].rearrange("p (h d) -> p h d", d=HS))
                else:
                    nc.vector.tensor_copy(dst, src)

            # ---- yT accumulator (written during attention) ----
            yT_s = ytpool.tile([128, KT, T], BF16, tag="yT")

            # prefetch projection weights (consumed only at the tail);
            # chained behind wv to keep early HBM bandwidth for x
            wpt = wppool.tile([128, KT, C], BF16, tag="wp")
            nc.vector.tensor_copy(wpt[0:1, KT - 1, C - 1:C],
                                  wv_s[0:1, KT - 1, C - 1:C])
            nc.sync.dma_start(wpt[:], wp_r)

            # ---- output projection, one t-tile unit at a time so
            # units can be interleaved into the last pair ----

            def emit_outproj_unit(it, on_scalar=False):
                ps = psA.tile([128, 1024], F32, tag="big",
                              name=f"ops_{it}")
                for off, w in ((0, 512), (512, 256)):
                    for k in range(KT):
                        nc.tensor.matmul(
                            ps[:, off:off + w],
                            yT_s[:, k, it * 128:(it + 1) * 128],
                            wpt[:, k, off:off + w],
                            start=(k == 0), stop=(k == KT - 1),
                        )
                ot = opool.tile([128, C], BF16, tag="ot",
                                name=f"ot_{it}")
                if bias_proj:
                    nc.vector.tensor_add(ot[:], ps[:, 0:C], bp_bc[:])
                elif on_scalar:
                    # tail units: ACT is idle after the last exp
                    nc.scalar.copy(ot[:], ps[:, 0:C])
                else:
                    nc.vector.tensor_copy(ot[:], ps[:, 0:C])
                nc.sync.dma_start(out_r[:, it, :], ot[:])

            # ---- attention core: S^T, exp, PV for one (pair, chunk) ----
            def emit_attn_core(hp, qk_t, c, filler=None, pre_g=None):
                njt = 4 * (c + 1)
                pts = [[ptpool.tile([128, 4, 512], BF16, tag="pt",
                                    name=f"pt_{hp}_{hl}_{c}_{i}")
                        for i in range(njt // 4)] for hl in range(2)]

                def pt_ap(hl, jt):
                    return pts[hl][jt // 4][:, jt % 4, :]

                y_pss = [psB.tile([128, 512], F32, tag="y",
                                  name=f"yps_{hp}_{hl}_{c}")
                         for hl in range(2)]

                def emit_pv(g):
                    # PV for the two j-tiles of group g (both heads)
                    for u in range(2):
                        jt = 2 * g + u
                        lo = max(0, (jt - 4 * c) * 128)
                        for hl in range(2):
                            nc.tensor.matmul(
                                y_pss[hl][:, lo:512],
                                v_aug[:, jt, 2 * hp + hl, :],
                                pt_ap(hl, jt)[:, lo:512],
                                start=(jt == 0),
                                stop=(jt == njt - 1),
                                skip_group_check=(jt > 0),
                            )

                # Software-pipelined emission: PV of group g-2 is
                # interleaved between S^T groups so the in-order PE
                # queue always has work that does not wait on the
                # ScalarE exp (which drains the S^T PSUM slots) and the
                # first PV lands after the previous chunk's norm frees
                # the y PSUM slot.
                LAG = 2
                for g in range(njt // 2):
                    if pre_g is not None and g in pre_g:
                        pre_g[g]()
                    sts = [psA.tile([128, 1024], F32, tag="big",
                                    name=f"st_{hp}_{hl}_{c}_{g}")
                           for hl in range(2)]
                    diag_g = 2 * g >= 4 * c
                    logp = max(0, (2 * g - 4 * c)) * 128
                    # heads sequential (not interleaved): head hl's exp
                    # is emitted before head hl+1's S^T so it starts ~1us
                    # earlier and its PSUM slot recycles sooner
                    for hl in range(2):
                        for u in range(2):
                            jt = 2 * g + u
                            lo = max(0, (jt - 4 * c) * 128)
                            nc.tensor.matmul(
                                sts[hl][:, u * 512 + lo:(u + 1) * 512],
                                qk_t[:, 1 + hl,
                                     jt * 128:(jt + 1) * 128],
                                qk_t[:, 0,
                                     c * 512 + lo:(c + 1) * 512],
                                start=True, stop=not diag_g,
                            )
                        if diag_g:
                            # accumulate -1920 on the j>i triangles of
                            # both diagonal blocks with one 256-col
                            # matmul: the u0 block sits at sub-block b0,
                            # the u1 block 5 sub-blocks later
                            b0 = 2 * g - 4 * c
                            dst = sts[hl][:].rearrange(
                                "p (x n) -> p x n", n=128)[:, b0:b0 + 6:5, :]
                            nc.tensor.matmul(
                                dst, id_s[:],
                                mtri2_s[:].rearrange("p (u n) -> p u n",
                                                     n=128),
                                start=False, stop=True,
                                skip_group_check=True,
                            )
                        # exp only the live columns [lo(2g):512] of both
                        # strips — columns left of the first j-tile's lo
                        # are never read by PV
                        nc.scalar.activation(
                            pts[hl][g // 2][:, (g % 2) * 2:(g % 2) * 2 + 2,
                                            logp:512],
                            sts[hl][:].rearrange("p (a n) -> p a n",
                                                 n=512)[:, :, logp:512],
                            mybir.ActivationFunctionType.Exp,
                            scale=0.125,
                        )
                    if g >= LAG:
                        emit_pv(g - LAG)
                    if filler is not None:
                        filler()
                for gg in range(max(0, njt // 2 - LAG), njt // 2):
                    emit_pv(gg)
                return y_pss

            # ---- normalize: yT[h rows, c cols] = y / sums ----
            # Stage 1 (DVE): yst copy frees the y PSUM slot; sums-row
            # copy from PSUM (shifts partition 64 -> 0; the SBUF->SBUF
            # shifted copy reads the wrong lane) and reciprocal in
            # place (custom-DVE needs base partition 0).
            # normalize: yT = y / sums, all on DVE, 64 lanes wide (the
            # PV matmul put 64 copies of the sums in PSUM rows 64-127).
            def emit_norm(hp, c, y_pss, stage_y=True):
                ysts, srows = [], []
                for hl in range(2):
                    if stage_y:
                        yst = smpool.tile([64, 512], F32, tag="yst",
                                          name=f"yst_{hp}_{hl}_{c}")
                        nc.vector.tensor_copy(yst[:], y_pss[hl][0:HS, :])
                    else:
                        yst = None
                    ysts.append(yst)
                for hl in range(2):
                    srow = smpool.tile([64, 512], F32, tag="srow",
                                       name=f"srow_{hp}_{hl}_{c}")
                    nc.vector.tensor_copy(srow[:], y_pss[hl][HS:128, :])
                    nc.vector.reciprocal_approx_fast(srow[:], srow[:])
                    srows.append(srow)
                for hl in range(2):
                    src = (ysts[hl][:] if ysts[hl] is not None
                           else y_pss[hl][0:HS, :])
                    nc.vector.tensor_mul(
                        yT_s[64 * hl:64 * hl + 64, hp,
                             c * 512:(c + 1) * 512],
                        src, srows[hl][:])

            # Output-projection units for t-tiles 0-3 only need chunk-0
            # yT columns (all pairs), so they can slot into the last
            # pair's chunk-1 emission; t-tiles 4-7 run at the tail.
            fill_iter = iter(range(4))

            def filler():
                u = next(fill_iter, None)
                if u is not None:
                    emit_outproj_unit(u)

            for hp in range(NPAIR):
                qk_t = qk_next
                last = hp == NPAIR - 1
                if not last:
                    wt_next = emit_qkproj_dma(hp + 1)
                y0 = emit_attn_core(hp, qk_t, 0)
                if not last:
                    qk_next = emit_qkproj(hp + 1, wt_next)

                    # norm of chunk 0 is emitted inside chunk 1 (g=1):
                    # if it were emitted first, its DVE ops would jump
                    # ahead of the parked qk copies in the DVE wait
                    # queue and delay the PSUM recycle.
                    def norm0(hp=hp, y0=y0):
                        emit_norm(hp, 0, y0)

                    y1 = emit_attn_core(hp, qk_t, 1, pre_g={1: norm0})
                else:
                    emit_norm(hp, 0, y0)
                    y1 = emit_attn_core(hp, qk_t, 1, filler=filler)
                emit_norm(hp, 1, y1, stage_y=not last)

            for u in fill_iter:
                emit_outproj_unit(u, on_scalar=True)
            for u in range(4, 8):
                emit_outproj_unit(u, on_scalar=True)

    nc.compile()
    return nc


def _get_program(bias_attn, bias_proj):
    key = (bias_attn, bias_proj)
    if key not in _cache:
        _cache[key] = _build_program(bias_attn, bias_proj)
    return _cache[key]


def _prep_inputs(x, w_attn, b_attn, w_proj, b_proj):
    x = np.asarray(x, dtype=np.float32)
    w_attn = np.asarray(w_attn, dtype=np.float32)
    b_attn = np.asarray(b_attn, dtype=np.float32)
    w_proj = np.asarray(w_proj, dtype=np.float32)
    b_proj = np.asarray(b_proj, dtype=np.float32)
    bias_attn = bool(np.any(b_attn))
    bias_proj = bool(np.any(b_proj))
    import ml_dtypes
    bf = ml_dtypes.bfloat16

    # Pre-tile to [128, free-contiguous] layouts (see _build_program).
    # wqk[p, hp, k, u, m] = w_attn[k*128+p, u*C + hp*128 + m]
    wqk = np.ascontiguousarray(
        w_attn[:, :2 * C].reshape(KT, 128, 2, NPAIR, 128)
        .transpose(1, 3, 0, 2, 4).reshape(128, -1)).astype(bf)
    # wv[p, k, m] = w_attn[k*128+p, 2C + m]
    wv = np.ascontiguousarray(
        w_attn[:, 2 * C:].reshape(KT, 128, C)
        .transpose(1, 0, 2).reshape(128, -1)).astype(bf)
    # wp[p, k, m] = w_proj[k*128+p, m]
    wpb = np.ascontiguousarray(
        w_proj.reshape(KT, 128, C).transpose(1, 0, 2).reshape(128, -1)
    ).astype(bf)
    in_maps = []
    for b in range(NCORES):
        # xT[p, k, t] = x[b, t, k*128+p]
        xt = np.ascontiguousarray(
            x[b].T.reshape(KT, 128, T).transpose(1, 0, 2).reshape(128, -1)
        ).astype(bf)
        m = {
            "xT": xt,
            "wqk": wqk,
            "wv": wv,
            "wp": wpb,
        }
        if bias_attn:
            m["bqk"] = np.ascontiguousarray(b_attn[:2 * C])
            m["bv"] = np.ascontiguousarray(b_attn[2 * C:])
        if bias_proj:
            m["bp"] = b_proj
        in_maps.append(m)
    return in_maps, bias_attn, bias_proj


def run(x, w_attn, b_attn, w_proj, b_proj, trace=False, tmpdir=None):
    in_maps, bias_attn, bias_proj = _prep_inputs(
        x, w_attn, b_attn, w_proj, b_proj)
    nc = _get_program(bias_attn, bias_proj)
    res = run_bass_kernel_spmd(nc, in_maps, list(range(NCORES)),
                               trace=trace, tmpdir=tmpdir)
    # out[p, it, m] -> [T, C]
    out = np.stack(
        [np.ascontiguousarray(
            res.results[i]["out"].reshape(128, 8, C).transpose(1, 0, 2)
            .reshape(T, C)) for i in range(NCORES)], axis=0)
    return out.astype(np.float32), res


def kernel(x, w_attn, b_attn, w_proj, b_proj):
    out, _ = run(x, w_attn, b_attn, w_proj, b_proj)
    return out
